# revision 1
# baseline (speedup 1.0000x reference)
"""Causal multi-head self-attention (RoPE) Trainium2 Bass kernel, 8-way
head-parallel.

Sharding: 16 heads / 8 cores = 2 heads per core (tensor parallel). Each core
receives the full (pre-transposed) activation matrix plus its head-slice of
w_qkv (with the RoPE interleave->half-split permutation folded into the
weight rows) and its 128-column slice of w_out. Each core computes a full
[8192, 1024] partial of the output projection; the host sums the 8 partials
(the all-reduce equivalent).

All matmul operands are pre-encoded on the host into the PE's FP32R format
(fp32 with the low 12 mantissa bits rounded away, RNE) so no on-chip
rounding passes are needed.

Per core:
  qkv^T = W @ x^T                 (M=384: Q,K,V rows; K accumulated in PSUM)
  u_sw  = SWAP @ u                (PE, 128x128 pair-swap matrix)
  rot   = u * cosA + u_sw * sinS  (RoPE on DVE; heads unstacked to base-0)
  S^T   = rot_k^T rot_q per 128-key tile; exp on ACT (scale=1/8) -> f32r
  causal: variable-N matmuls skip below-diagonal tiles; gpsimd affine_select
          zeroes the strict lower triangle of diagonal blocks
  PV    = [V | 1]^T @ E^T         (ones column gives the softmax denominator)
  out^T = PV[0:64] * recip(bcast(PV[64]))   (PE ones-row broadcast + DVE)
  y     = out^T^T @ w_out_slice^T (K=128, single matmul per tile)
"""
import os
import sys

for _p in ("/opt/trn_rl_repo", "/root/.axon_site/_ro/trn_rl_repo"):
    if os.path.isdir(_p) and _p not in sys.path:
        sys.path.insert(0, _p)

import numpy as np

B, S, D_MODEL, N_HEADS, D_HEAD = 4, 2048, 1024, 16, 64
N_CORES, H_PER = 8, 2
THETA = 10000.0
BS = B * S
KC = D_MODEL // 128             # 8 contraction chunks
NQ = S // 512                   # 4 query chunks per batch
NK = S // 128                   # 16 key tiles per batch

_PERM = np.concatenate([np.arange(0, 64, 2), np.arange(1, 64, 2)])
_INVF = THETA ** (-np.arange(32) * 2.0 / 64)

_cached = {}
TRACE = False            # set True to capture an NTFF profile on the next run
LAST_EXEC_NS = None      # max-core HW exec time of the last traced run
LAST_TRACE_PATH = None


def f32r_encode(a):
    """Round fp32 to the PE's FP32R format: RNE drop of low 12 mantissa bits."""
    xb = np.ascontiguousarray(a, np.float32).view(np.uint32).astype(np.uint64)
    low = xb & 0xFFF
    base = xb >> 12
    add = (low > 0x800) | ((low == 0x800) & ((base & 1) == 1))
    out = (((base + add) << 12) & 0xFFFFFFFF).astype(np.uint32)
    return out.view(np.float32).reshape(a.shape)


def _host_tables():
    pos = np.arange(S, dtype=np.float64)
    ang = pos[None, :] * _INVF[:, None]
    cosb, sinb = np.cos(ang), np.sin(ang)
    cosA64 = np.concatenate([cosb, cosb], 0)
    sinS64 = np.concatenate([-sinb, sinb], 0)
    cosA = np.concatenate([cosA64, cosA64], 0).astype(np.float32)   # [128, S]
    sinS = np.concatenate([sinS64, sinS64], 0).astype(np.float32)
    sw64 = np.zeros((64, 64), np.float64)
    sw64[:32, 32:] = np.eye(32)
    sw64[32:, :32] = np.eye(32)
    SW = np.block([[sw64, np.zeros((64, 64))], [np.zeros((64, 64)), sw64]])
    return cosA, sinS, SW


def _host_prep(x, w_qkv, w_out):
    cosA, sinS, SW = _host_tables()
    xT_r = f32r_encode(x.reshape(BS, D_MODEL).T)
    swap_r = f32r_encode(SW.astype(np.float32))
    in_maps = []
    for c in range(N_CORES):
        h0, h1 = 2 * c, 2 * c + 1
        wq = np.concatenate([w_qkv[64 * h0:64 * h0 + 64][_PERM],
                             w_qkv[64 * h1:64 * h1 + 64][_PERM]], 0)
        wk = np.concatenate([w_qkv[1024 + 64 * h0:1024 + 64 * h0 + 64][_PERM],
                             w_qkv[1024 + 64 * h1:1024 + 64 * h1 + 64][_PERM]], 0)
        wv = np.concatenate([w_qkv[2048 + 64 * h0:2048 + 64 * h0 + 64],
                             w_qkv[2048 + 64 * h1:2048 + 64 * h1 + 64]], 0)
        w_all = np.concatenate([wq, wk, wv], 0)          # [384, 1024]
        in_maps.append({
            "xT": xT_r,
            "wqkvT": f32r_encode(w_all.T),
            "woutT": f32r_encode(w_out[:, 128 * c:128 * c + 128].T),
            "swapm": swap_r,
            "cosA": cosA,
            "sinS": sinS,
        })
    return in_maps


def _build_nc():
    import concourse.bacc as bacc
    import concourse.mybir as mybir
    from concourse import tile
    from concourse.masks import make_identity

    F32, F32R = mybir.dt.float32, mybir.dt.float32r
    AF = mybir.ActivationFunctionType
    ALU = mybir.AluOpType

    nc = bacc.Bacc("TRN2", target_bir_lowering=False, debug=False,
                   num_devices=N_CORES)
    xT_d = nc.dram_tensor("xT", [D_MODEL, BS], F32R, kind="ExternalInput")
    w_d = nc.dram_tensor("wqkvT", [D_MODEL, 384], F32R, kind="ExternalInput")
    wo_d = nc.dram_tensor("woutT", [128, D_MODEL], F32R, kind="ExternalInput")
    sw_d = nc.dram_tensor("swapm", [128, 128], F32R, kind="ExternalInput")
    cos_d = nc.dram_tensor("cosA", [128, S], F32, kind="ExternalInput")
    sin_d = nc.dram_tensor("sinS", [128, S], F32, kind="ExternalInput")
    y_d = nc.dram_tensor("y", [BS, D_MODEL], F32, kind="ExternalOutput")

    with tile.TileContext(nc) as tc:
        with tc.tile_pool(name="const", bufs=1) as const, \
             tc.tile_pool(name="xr", bufs=2) as xrp, \
             tc.tile_pool(name="ur", bufs=3) as urp, \
             tc.tile_pool(name="ropet", bufs=2) as ropet, \
             tc.tile_pool(name="rot", bufs=2) as rotp, \
             tc.tile_pool(name="vr", bufs=1) as vrp, \
             tc.tile_pool(name="vone", bufs=2) as vonep, \
             tc.tile_pool(name="ep", bufs=4) as ep, \
             tc.tile_pool(name="outp", bufs=2) as outp, \
             tc.tile_pool(name="nrm", bufs=2) as nrm, \
             tc.tile_pool(name="ysb", bufs=3) as ysb, \
             tc.tile_pool(name="ps", bufs=1, space="PSUM") as ps:

            # ---- constants -------------------------------------------------
            w_r = const.tile([128, KC, 384], F32R, tag="w")
            nc.sync.dma_start(
                w_r[:], w_d.ap().rearrange("(kc p) m -> p kc m", p=128))
            wo_r = const.tile([128, D_MODEL], F32R, tag="wo")
            nc.sync.dma_start(wo_r[:], wo_d.ap())
            swap_r = const.tile([128, 128], F32R, tag="swap")
            nc.sync.dma_start(swap_r[:], sw_d.ap())
            cosA = const.tile([128, S], F32, tag="cos")
            sinS = const.tile([128, S], F32, tag="sin")
            nc.sync.dma_start(cosA[:], cos_d.ap())
            nc.sync.dma_start(sinS[:], sin_d.ap())
            idf = const.tile([128, 128], F32, tag="idf")
            make_identity(nc, idf[:])
            id128_r = const.tile([128, 128], F32R, tag="idr")
            nc.vector.tensor_copy(id128_r[:], idf[:])
            onef = const.tile([128, 1], F32, tag="onef")
            nc.vector.memset(onef[:], 1.0)
            one_r = const.tile([128, 1], F32R, tag="oner")
            nc.vector.tensor_copy(one_r[:], onef[:])
            orow_f = const.tile([1, 64], F32, tag="orowf")
            nc.vector.memset(orow_f[:], 1.0)
            orow_r = const.tile([1, 64], F32R, tag="orow")
            nc.vector.tensor_copy(orow_r[:], orow_f[:])

            for b in range(B):
                c0 = b * S

                # ---- QKV projection + RoPE + V transpose -------------------
                rot_q = rotp.tile([64, 2 * S], F32R, tag="rq")
                rot_k = rotp.tile([64, 2 * S], F32R, tag="rk")
                v_r = vrp.tile([128, S], F32R, tag="vr")
                vone = vonep.tile([128, H_PER, NK, 65], F32R, tag="vone")

                for n in range(NQ):
                    nsl = slice(n * 512, (n + 1) * 512)
                    x_r = xrp.tile([128, KC, 512], F32R, tag="xr")
                    for half in range(2):
                        nc.sync.dma_start(
                            x_r[:, 4 * half:4 * half + 4, :],
                            xT_d.ap()[512 * half:512 * half + 512,
                                      c0 + n * 512:c0 + (n + 1) * 512]
                            .rearrange("(kc p) n -> p kc n", p=128))

                    # projection chains first (PE stays busy), swaps
                    # interleaved so PE never waits on the ACT evictions
                    pus, urs = [], []
                    for qk in range(2):        # 0: Q, 1: K
                        pu = ps.tile([128, 512], F32, tag="a", bufs=4)
                        for kc in range(KC):
                            nc.tensor.matmul(
                                pu[:], w_r[:, kc, 128 * qk:128 * qk + 128],
                                x_r[:, kc, :], start=(kc == 0),
                                stop=(kc == KC - 1))
                        u_r = urp.tile([128, 512], F32R, tag="ur")
                        nc.scalar.copy(u_r[:], pu[:])
                        pus.append(pu)
                        urs.append(u_r)
                    pvv = ps.tile([128, 512], F32, tag="v", bufs=2)
                    for kc in range(KC):
                        nc.tensor.matmul(pvv[:], w_r[:, kc, 256:384],
                                         x_r[:, kc, :], start=(kc == 0),
                                         stop=(kc == KC - 1))
                    nc.scalar.copy(v_r[:, n * 512:(n + 1) * 512], pvv[:])
                    for qk in range(2):
                        u_r = urs[qk]
                        psw = ps.tile([128, 512], F32, tag="a", bufs=4)
                        nc.tensor.matmul(psw[:], swap_r[:], u_r[:],
                                         start=True, stop=True)
                        t_sb = ropet.tile([128, 512], F32, tag="t")
                        nc.vector.tensor_mul(t_sb[:], psw[:], sinS[:, nsl])
                        m_sb = ropet.tile([128, 512], F32, tag="m")
                        nc.vector.tensor_mul(m_sb[:], u_r[:], cosA[:, nsl])
                        rot = rot_q if qk == 0 else rot_k
                        for hh in range(H_PER):
                            nc.vector.tensor_add(
                                rot[:, hh * S + n * 512: hh * S + (n + 1) * 512],
                                t_sb[64 * hh:64 * hh + 64, :],
                                m_sb[64 * hh:64 * hh + 64, :])

                for ki in range(NK):
                    pt = ps.tile([128, 128], F32R, tag="v", bufs=2)
                    nc.tensor.transpose(
                        pt[:], v_r[:, ki * 128:(ki + 1) * 128], id128_r[:])
                    for hh in range(H_PER):
                        nc.vector.tensor_copy(vone[:, hh, ki, :64],
                                              pt[:, 64 * hh:64 * hh + 64])
                        nc.gpsimd.tensor_copy(vone[:, hh, ki, 64:65], one_r[:])

                # ---- attention --------------------------------------------
                outT = outp.tile([128, S], F32R, tag="outT")
                for hh in range(H_PER):
                    for qc in range(NQ):
                        pv = ps.tile([65, 512], F32, tag="pv", bufs=2)
                        last_ki = 4 * qc + 3
                        pending = []   # (e_t, nc_cols, ki) awaiting their PV
                        for ki in range(last_ki + 1):
                            nc_cols = 512 - max(0, ki - 4 * qc) * 128
                            coff = max(qc * 512, ki * 128)
                            st = ps.tile([128, 512], F32, tag="a", bufs=4)
                            nc.tensor.matmul(
                                st[:, :nc_cols],
                                rot_k[:, hh * S + ki * 128: hh * S + (ki + 1) * 128],
                                rot_q[:, hh * S + coff: hh * S + coff + nc_cols],
                                start=True, stop=True)
                            e_t = ep.tile([128, 512], F32R, tag="e")
                            nc.scalar.activation(e_t[:, :nc_cols], st[:, :nc_cols],
                                                 AF.Exp, scale=0.125)
                            if ki >= 4 * qc:   # diagonal block: zero k > q
                                nc.gpsimd.affine_select(
                                    out=e_t[:, :128], in_=e_t[:, :128],
                                    compare_op=ALU.is_ge, fill=0.0,
                                    base=0, pattern=[[1, 128]],
                                    channel_multiplier=-1)
                            pending.append((e_t, nc_cols, ki))
                            if len(pending) > 1:
                                pe_t, pnc, pki = pending.pop(0)
                                nc.tensor.matmul(
                                    pv[:, 512 - pnc:], vone[:, hh, pki, :],
                                    pe_t[:, :pnc], start=(pki == 0),
                                    stop=(pki == last_ki))
                        for pe_t, pnc, pki in pending:
                            nc.tensor.matmul(
                                pv[:, 512 - pnc:], vone[:, hh, pki, :],
                                pe_t[:, :pnc], start=(pki == 0),
                                stop=(pki == last_ki))

                        den_r = nrm.tile([1, 512], F32R, tag="den")
                        nc.scalar.copy(den_r[:], pv[64:65, :])
                        pbc = ps.tile([64, 512], F32, tag="v", bufs=2)
                        nc.tensor.matmul(pbc[:], orow_r[:], den_r[:],
                                         start=True, stop=True)
                        rb = nrm.tile([64, 512], F32, tag="rb")
                        nc.vector.reciprocal(rb[:], pbc[:])
                        nc.vector.tensor_mul(
                            outT[64 * hh:64 * hh + 64, qc * 512:(qc + 1) * 512],
                            pv[:64, :], rb[:])

                # ---- output projection ------------------------------------
                for t in range(16):
                    for ec in range(2):
                        py = ps.tile([128, 512], F32, tag="pv", bufs=2)
                        nc.tensor.matmul(py[:],
                                         outT[:, t * 128:(t + 1) * 128],
                                         wo_r[:, ec * 512:(ec + 1) * 512],
                                         start=True, stop=True)
                        y_sb = ysb.tile([128, 512], F32, tag="ysb")
                        nc.vector.tensor_copy(y_sb[:], py[:])
                        nc.sync.dma_start(
                            y_d.ap()[c0 + t * 128: c0 + (t + 1) * 128,
                                     ec * 512:(ec + 1) * 512],
                            y_sb[:])
    nc.compile()
    return nc


def _get_nc():
    if "nc" not in _cached:
        _cached["nc"] = _build_nc()
    return _cached["nc"]


def kernel(x, w_qkv, w_out):
    from concourse.bass_utils import run_bass_kernel_spmd

    x = np.asarray(x, np.float32)
    w_qkv = np.asarray(w_qkv, np.float32)
    w_out = np.asarray(w_out, np.float32)
    in_maps = _host_prep(x, w_qkv, w_out)
    nc = _get_nc()
    res = run_bass_kernel_spmd(nc, in_maps, core_ids=list(range(N_CORES)),
                               trace=TRACE)
    global LAST_EXEC_NS, LAST_TRACE_PATH
    if res.exec_time_ns is not None:
        LAST_EXEC_NS = res.exec_time_ns
        if res.instructions_and_trace:
            LAST_TRACE_PATH = res.instructions_and_trace[1]
    y = np.sum(np.stack([res.results[c]["y"] for c in range(N_CORES)]),
               axis=0, dtype=np.float64)
    return y.reshape(B, S, D_MODEL).astype(np.float32)



# revision 13
# speedup vs baseline: 1.2747x; 1.2747x over previous
"""Causal multi-head self-attention (RoPE) Trainium2 Bass kernel, 8-way
head-parallel, fp16 datapath.

Sharding: 16 heads / 8 cores = 2 heads per core (tensor parallel). Each core
receives the full (pre-transposed) activation matrix plus its head-slice of
w_qkv (with the RoPE interleave->16-block permutation folded into the weight
rows) and its 128-column slice of w_out. Each core computes a full
[1024, 8192] y^T partial of the output projection; the host sums the 8
partials (the all-reduce equivalent) and transposes.

All matmul operands are fp16: full-rate PE (1 cycle/row), fast weight load
(FWL fires for 2-byte dtypes), half DMA/SBUF vs fp32.

Per core:
  qkv^T = W @ x^T            (M=384 rows; K accumulated in PSUM, 2-bank pass)
  u_sw  = stream_shuffle(u)  (DVE 16-block swap inside 32-partition quadrants;
                              the RoPE pair permutation is folded into W rows)
  rot   = u*cosA + u_sw*sinS (DVE, heads stacked on partitions)
  S^T   = rot_k^T rot_q per 128-key tile (tile_position selects head band);
          exp on ACT (scale=1/8) -> fp16; causal: variable-N matmuls skip
          below-diagonal tiles; gpsimd affine_select zeroes the strict lower
          triangle of diagonal blocks
  PV    = [V | 1]^T @ E^T    (ones column gives the softmax denominator)
  den   : DVE reciprocal from PSUM row 64 -> PE broadcast into rows 64:128 of
          the same PSUM bank -> DVE mul gives normalized out^T (fp16)
  y^T   = w_out_chunk^T @ out^T  (w_out stationary, 8 loads/batch)

Cross-batch software pipelining: QKV(b+1) PE work is interleaved into
attention(b)'s instruction stream so the PE never idles on the ACT exp
round-trip.
"""
import os
import sys

for _p in ("/opt/trn_rl_repo", "/root/.axon_site/_ro/trn_rl_repo"):
    if os.path.isdir(_p) and _p not in sys.path:
        sys.path.insert(0, _p)

import numpy as np

B, S, D_MODEL, N_HEADS, D_HEAD = 4, 2048, 1024, 16, 64
N_CORES, H_PER = 8, 2
THETA = 10000.0
BS = B * S
KC = D_MODEL // 128             # 8 contraction chunks
NQ = S // 512                   # 4 query chunks per batch
NK = S // 128                   # 16 key tiles per batch

# RoPE pair layout: per 64-row head block, 16-blocks
#   [x1_{0..15} | x2_{0..15} | x1_{16..31} | x2_{16..31}]
# so the rotate-half swap is a 16-block swap INSIDE each 32-partition
# quadrant, expressible as a DVE stream_shuffle.
_PERM = np.concatenate([np.arange(0, 32, 2), np.arange(1, 32, 2),
                        np.arange(32, 64, 2), np.arange(33, 64, 2)])
_INVF = THETA ** (-np.arange(32) * 2.0 / 64)
_SHUF_MASK = list(range(16, 32)) + list(range(0, 16))

_cached = {}
TRACE = False            # set True to capture an NTFF profile on the next run
LAST_EXEC_NS = None      # max-core HW exec time of the last traced run
LAST_TRACE_PATH = None


def _host_tables():
    pos = np.arange(S, dtype=np.float64)
    ang = pos[None, :] * _INVF[:, None]          # [32, S]
    cosb, sinb = np.cos(ang), np.sin(ang)
    cos64 = np.concatenate([cosb[0:16], cosb[0:16], cosb[16:32], cosb[16:32]], 0)
    sin64 = np.concatenate([-sinb[0:16], sinb[0:16], -sinb[16:32], sinb[16:32]], 0)
    cosA = np.concatenate([cos64, cos64], 0).astype(np.float16)   # [128, S]
    sinS = np.concatenate([sin64, sin64], 0).astype(np.float16)
    return cosA, sinS


def _host_prep(x, w_qkv, w_out):
    cosA, sinS = _host_tables()
    xT = np.ascontiguousarray(x.reshape(BS, D_MODEL).T).astype(np.float16)
    in_maps = []
    for c in range(N_CORES):
        h0, h1 = 2 * c, 2 * c + 1
        wq = np.concatenate([w_qkv[64 * h0:64 * h0 + 64][_PERM],
                             w_qkv[64 * h1:64 * h1 + 64][_PERM]], 0)
        wk = np.concatenate([w_qkv[1024 + 64 * h0:1024 + 64 * h0 + 64][_PERM],
                             w_qkv[1024 + 64 * h1:1024 + 64 * h1 + 64][_PERM]], 0)
        wv = np.concatenate([w_qkv[2048 + 64 * h0:2048 + 64 * h0 + 64],
                             w_qkv[2048 + 64 * h1:2048 + 64 * h1 + 64]], 0)
        w_all = np.concatenate([wq, wk, wv], 0)          # [384, 1024]
        in_maps.append({
            "xT": xT,
            "wqkvT": np.ascontiguousarray(w_all.T).astype(np.float16),
            "woutT": np.ascontiguousarray(
                w_out[:, 128 * c:128 * c + 128].T).astype(np.float16),
            "cosA": cosA,
            "sinS": sinS,
        })
    return in_maps


def _build_nc():
    import concourse.bacc as bacc
    import concourse.mybir as mybir
    from concourse import tile
    from concourse.masks import make_identity

    F16, F32, F32R = mybir.dt.float16, mybir.dt.float32, mybir.dt.float32r
    AF = mybir.ActivationFunctionType
    ALU = mybir.AluOpType

    nc = bacc.Bacc("TRN2", target_bir_lowering=False, debug=False,
                   num_devices=N_CORES)
    xT_d = nc.dram_tensor("xT", [D_MODEL, BS], F16, kind="ExternalInput")
    w_d = nc.dram_tensor("wqkvT", [D_MODEL, 384], F16, kind="ExternalInput")
    wo_d = nc.dram_tensor("woutT", [128, D_MODEL], F16, kind="ExternalInput")
    cos_d = nc.dram_tensor("cosA", [128, S], F16, kind="ExternalInput")
    sin_d = nc.dram_tensor("sinS", [128, S], F16, kind="ExternalInput")
    y_d = nc.dram_tensor("y", [D_MODEL, BS], F16, kind="ExternalOutput")

    with tile.TileContext(nc) as tc:
        with tc.tile_pool(name="const", bufs=1) as const, \
             tc.tile_pool(name="xr", bufs=3) as xrp, \
             tc.tile_pool(name="ur", bufs=3) as urp, \
             tc.tile_pool(name="ropet", bufs=2) as ropet, \
             tc.tile_pool(name="rot", bufs=2) as rotp, \
             tc.tile_pool(name="vr", bufs=2) as vrp, \
             tc.tile_pool(name="vone", bufs=2) as vonep, \
             tc.tile_pool(name="ep", bufs=6) as ep, \
             tc.tile_pool(name="outp", bufs=2) as outp, \
             tc.tile_pool(name="nrm", bufs=2) as nrm, \
             tc.tile_pool(name="ysb", bufs=3) as ysb, \
             tc.tile_pool(name="ps", bufs=1, space="PSUM") as ps:

            # ---- constants -------------------------------------------------
            w_r = const.tile([128, KC, 384], F16, tag="w")
            nc.sync.dma_start(
                w_r[:], w_d.ap().rearrange("(kc p) m -> p kc m", p=128))
            wo_r = const.tile([128, D_MODEL], F16, tag="wo")
            nc.sync.dma_start(wo_r[:], wo_d.ap())
            cosA = const.tile([128, S], F16, tag="cos")
            sinS = const.tile([128, S], F16, tag="sin")
            nc.sync.dma_start(cosA[:], cos_d.ap())
            nc.sync.dma_start(sinS[:], sin_d.ap())
            idf = const.tile([128, 128], F32, tag="idf")
            make_identity(nc, idf[:])
            id128 = const.tile([128, 128], F16, tag="idr")
            nc.vector.tensor_copy(id128[:], idf[:])
            orow_f = const.tile([1, 64], F32, tag="orowf")
            nc.vector.memset(orow_f[:], 1.0)
            orow_r = const.tile([1, 64], F16, tag="orow")
            nc.vector.tensor_copy(orow_r[:], orow_f[:])

            # Per-batch persistent tiles, allocated per batch via pools:
            rot_tiles = {}      # (b % 2 handled by pool rotation)
            vone_tiles = {}
            outT_tiles = {}

            def gen_qkv(b):
                """QKV projection + RoPE + V transpose for batch b.
                Yields at chunk boundaries for interleaving."""
                c0 = b * S
                rot_q = rotp.tile([128, S], F16, tag="rq")
                rot_k = rotp.tile([128, S], F16, tag="rk")
                v_r = vrp.tile([128, S], F16, tag="vr")
                vone = vonep.tile([128, H_PER, NK, 65], F16, tag="vone")
                rot_tiles[b] = (rot_q, rot_k)
                vone_tiles[b] = vone
                nc.gpsimd.memset(vone[:, :, :, 64:65], 1.0)

                for half in range(2):           # n-chunk pairs (0,1), (2,3)
                    x_rs = []
                    for nn in (2 * half, 2 * half + 1):
                        x_r = xrp.tile([128, KC, 512], F16, tag="xr")
                        for hf in range(2):
                            nc.sync.dma_start(
                                x_r[:, 4 * hf:4 * hf + 4, :],
                                xT_d.ap()[512 * hf:512 * hf + 512,
                                          c0 + nn * 512:c0 + (nn + 1) * 512]
                                .rearrange("(kc p) n -> p kc n", p=128))
                        x_rs.append(x_r)
                    yield

                    # Q then K chains (2 PSUM banks, stationary reused
                    # across the bank pair per kc step)
                    for qk in range(2):
                        pus = [ps.tile([128, 512], F32, tag="qkv", bufs=2,
                                       name=f"pu{j}")
                               for j in range(2)]
                        for kc in range(KC):
                            for j in range(2):
                                nc.tensor.matmul(
                                    pus[j][:],
                                    w_r[:, kc, 128 * qk:128 * qk + 128],
                                    x_rs[j][:, kc, :], start=(kc == 0),
                                    stop=(kc == KC - 1))
                            if kc == 3:
                                yield
                        yield
                        # evict + RoPE per n-chunk
                        rot = rot_q if qk == 0 else rot_k
                        for j in range(2):
                            nn = 2 * half + j
                            nsl = slice(nn * 512, (nn + 1) * 512)
                            u_r = urp.tile([128, 512], F16, tag="ur")
                            nc.vector.tensor_copy(u_r[:], pus[j][:])
                            psw = ropet.tile([128, 512], F16, tag="sw")
                            nc.vector.stream_shuffle(psw[:], u_r[:], _SHUF_MASK)
                            m_sb = ropet.tile([128, 512], F16, tag="m")
                            nc.vector.tensor_mul(m_sb[:], u_r[:], cosA[:, nsl])
                            t_sb = ropet.tile([128, 512], F16, tag="t")
                            nc.vector.tensor_mul(t_sb[:], psw[:], sinS[:, nsl])
                            nc.vector.tensor_add(rot[:, nsl], m_sb[:], t_sb[:])
                        yield

                    # V chain
                    pvs = [ps.tile([128, 512], F32, tag="qkv", bufs=2,
                                   name=f"pvv{j}")
                           for j in range(2)]
                    for kc in range(KC):
                        for j in range(2):
                            nc.tensor.matmul(
                                pvs[j][:], w_r[:, kc, 256:384],
                                x_rs[j][:, kc, :], start=(kc == 0),
                                stop=(kc == KC - 1))
                        if kc == 3:
                            yield
                    yield
                    for j in range(2):
                        nn = 2 * half + j
                        nc.vector.tensor_copy(v_r[:, nn * 512:(nn + 1) * 512],
                                              pvs[j][:])
                    yield
                    # V transposes for these 8 key tiles
                    for kt in range(8 * half, 8 * half + 8):
                        pt = ps.tile([128, 128], F16, tag="py", bufs=2)
                        nc.tensor.transpose(
                            pt[:], v_r[:, kt * 128:(kt + 1) * 128], id128[:])
                        for hh in range(H_PER):
                            nc.vector.tensor_copy(
                                vone[:, hh, kt, :64],
                                pt[:, 64 * hh:64 * hh + 64])
                        if kt % 4 == 3:
                            yield

            def gen_attn(b):
                """Attention for batch b. Yields per ki step."""
                rot_q, rot_k = rot_tiles[b]
                vone = vone_tiles[b]
                outT = outp.tile([128, S], F16, tag="outT")
                outT_tiles[b] = outT

                for qc in range(NQ):
                    last_ki = 4 * qc + 3
                    pvt = {hh: ps.tile([128, 512], F32, tag="pv", bufs=2,
                                       name=f"pv{hh}")
                           for hh in range(H_PER)}
                    pending = []   # (hh, ki, e_t, nc_cols)
                    for ki in range(last_ki + 1):
                        nc_cols = 512 - max(0, ki - 4 * qc) * 128
                        coff = max(qc * 512, ki * 128)
                        for hh in range(H_PER):
                            st = ps.tile([128, 512], F32, tag="st", bufs=2)
                            nc.tensor.matmul(
                                st[:, :nc_cols],
                                rot_k[64 * hh:64 * hh + 64,
                                      ki * 128:(ki + 1) * 128],
                                rot_q[64 * hh:64 * hh + 64,
                                      coff:coff + nc_cols],
                                start=True, stop=True,
                                tile_position=(64 * hh, 0))
                            e_t = ep.tile([128, 512], F16, tag="e")
                            nc.scalar.activation(e_t[:, :nc_cols],
                                                 st[:, :nc_cols],
                                                 AF.Exp, scale=0.125)
                            if ki >= 4 * qc:   # diagonal block: zero k > q
                                nc.gpsimd.affine_select(
                                    out=e_t[:, :128], in_=e_t[:, :128],
                                    compare_op=ALU.is_ge, fill=0.0,
                                    base=0, pattern=[[1, 128]],
                                    channel_multiplier=-1)
                            pending.append((hh, ki, e_t, nc_cols))
                            while len(pending) > 2:
                                phh, pki, pe_t, pnc = pending.pop(0)
                                nc.tensor.matmul(
                                    pvt[phh][0:65, 512 - pnc:],
                                    vone[:, phh, pki, :],
                                    pe_t[:, :pnc], start=(pki == 0),
                                    stop=(pki == last_ki))
                        yield
                    for phh, pki, pe_t, pnc in pending:
                        nc.tensor.matmul(
                            pvt[phh][0:65, 512 - pnc:],
                            vone[:, phh, pki, :],
                            pe_t[:, :pnc], start=(pki == 0),
                            stop=(pki == last_ki))

                    # normalize: den row -> SBUF -> PE broadcast into rows
                    # 64:128 of the pv bank -> DVE recip -> fp16 out^T
                    for hh in range(H_PER):
                        pv = pvt[hh]
                        den1 = nrm.tile([1, 512], F16, tag="den")
                        nc.vector.tensor_copy(den1[:], pv[64:65, :])
                        nc.tensor.matmul(pv[64:128, :], orow_r[:], den1[:],
                                         start=True, stop=True,
                                         tile_position=(0, 64),
                                         skip_group_check=True)
                        rb = nrm.tile([64, 512], F16, tag="rb")
                        with nc.allow_low_precision(
                                reason="fp16 softmax denominators"):
                            nc.vector.reciprocal(rb[:], pv[64:128, :])
                        nc.vector.tensor_mul(
                            outT[64 * hh:64 * hh + 64,
                                 qc * 512:(qc + 1) * 512],
                            pv[0:64, :], rb[:])
                    yield

            def gen_out(b):
                """Output projection y^T = wo^T @ out^T for batch b."""
                c0 = b * S
                outT = outT_tiles.pop(b)
                for ec in range(8):
                    for t in range(NQ):
                        py = ps.tile([128, 512], F32, tag="py", bufs=2)
                        nc.tensor.matmul(py[:],
                                         wo_r[:, ec * 128:(ec + 1) * 128],
                                         outT[:, t * 512:(t + 1) * 512],
                                         start=True, stop=True)
                        y_sb = ysb.tile([128, 512], F16, tag="ysb")
                        if (ec * NQ + t) % 2 == 0:
                            nc.vector.tensor_copy(y_sb[:], py[:])
                        else:
                            nc.scalar.copy(y_sb[:], py[:])
                        nc.sync.dma_start(
                            y_d.ap()[ec * 128:(ec + 1) * 128,
                                     c0 + t * 512:c0 + (t + 1) * 512],
                            y_sb[:])
                    yield
                rot_tiles.pop(b)
                vone_tiles.pop(b)

            def drain(g):
                for _ in g:
                    pass

            def interleave(main, filler):
                main, filler = iter(main), iter(filler)
                sentinel = object()
                while True:
                    m = next(main, sentinel)
                    f = next(filler, sentinel)
                    if m is sentinel and f is sentinel:
                        break

            import itertools
            drain(gen_qkv(0))
            for b in range(B):
                main = itertools.chain(gen_attn(b), gen_out(b))
                filler = gen_qkv(b + 1) if b + 1 < B else iter(())
                interleave(main, filler)
    nc.compile()
    return nc


def _get_nc():
    if "nc" not in _cached:
        _cached["nc"] = _build_nc()
    return _cached["nc"]


def kernel(x, w_qkv, w_out):
    from concourse.bass_utils import run_bass_kernel_spmd

    x = np.asarray(x, np.float32)
    w_qkv = np.asarray(w_qkv, np.float32)
    w_out = np.asarray(w_out, np.float32)
    in_maps = _host_prep(x, w_qkv, w_out)
    nc = _get_nc()
    res = run_bass_kernel_spmd(nc, in_maps, core_ids=list(range(N_CORES)),
                               trace=TRACE)
    global LAST_EXEC_NS, LAST_TRACE_PATH
    if res.exec_time_ns is not None:
        LAST_EXEC_NS = res.exec_time_ns
        if res.instructions_and_trace:
            LAST_TRACE_PATH = res.instructions_and_trace[1]
    yT = np.sum(np.stack([res.results[c]["y"].astype(np.float32)
                          for c in range(N_CORES)]), axis=0)
    return yT.T.reshape(B, S, D_MODEL).astype(np.float32)


# revision 14
# speedup vs baseline: 1.3626x; 1.0690x over previous
"""Causal multi-head self-attention (RoPE) Trainium2 Bass kernel, 8-way
head-parallel, fp16 datapath.

Sharding: 16 heads / 8 cores = 2 heads per core (tensor parallel). Each core
receives the full (pre-transposed) activation matrix plus its head-slice of
w_qkv (with the RoPE pair permutation folded into the weight rows) and its
128-column slice of w_out. Each core computes a full [1024, 8192] y^T partial
of the output projection; the host sums the 8 partials (the all-reduce
equivalent) and transposes.

All matmul operands are fp16 (full-rate PE, fast weight load, half DMA);
PSUM accumulates fp32.

Structure per core:
  qkv^T = W @ x^T            (PSUM pair-banks, stationary reused)
  u_sw  = stream_shuffle(u)  (DVE 16-block swap inside 32-partition quadrants;
                              the RoPE pair permutation is folded into W rows)
  rot   = u*cosA + u_sw*sinS (full-width [128, 2048] DVE ops, q/k interleaved)
  S^T per ki for BOTH heads into one 2-bank PSUM pair-tile; ONE exp per ki
  PV    = [V | 1]^T @ E^T    (ones column gives the softmax denominator)
  den   -> SBUF -> PE broadcast into rows 64:128 of the pv bank -> DVE recip
        -> DVE mul -> fp16 out^T (chains for the two heads stage-interleaved)
  y^T   = w_out_chunk^T @ out^T into pair-banks; one [128,1024] eviction per
          pair, alternating DVE/ACT

Cross-batch software pipelining: QKV(b+1) work is interleaved into
attention(b)'s instruction stream so the PE never idles on the ACT exp
round-trip.
"""
import os
import sys

for _p in ("/opt/trn_rl_repo", "/root/.axon_site/_ro/trn_rl_repo"):
    if os.path.isdir(_p) and _p not in sys.path:
        sys.path.insert(0, _p)

import numpy as np

B, S, D_MODEL, N_HEADS, D_HEAD = 4, 2048, 1024, 16, 64
N_CORES, H_PER = 8, 2
THETA = 10000.0
BS = B * S
KC = D_MODEL // 128             # 8 contraction chunks
NQ = S // 512                   # 4 query chunks per batch
NK = S // 128                   # 16 key tiles per batch

# RoPE pair layout: per 64-row head block, 16-blocks
#   [x1_{0..15} | x2_{0..15} | x1_{16..31} | x2_{16..31}]
# so the rotate-half swap is a 16-block swap INSIDE each 32-partition
# quadrant, expressible as a DVE stream_shuffle.
_PERM = np.concatenate([np.arange(0, 32, 2), np.arange(1, 32, 2),
                        np.arange(32, 64, 2), np.arange(33, 64, 2)])
_INVF = THETA ** (-np.arange(32) * 2.0 / 64)
_SHUF_MASK = list(range(16, 32)) + list(range(0, 16))

_cached = {}
TRACE = False            # set True to capture an NTFF profile on the next run
LAST_EXEC_NS = None      # max-core HW exec time of the last traced run
LAST_TRACE_PATH = None


def _host_tables():
    pos = np.arange(S, dtype=np.float64)
    ang = pos[None, :] * _INVF[:, None]          # [32, S]
    cosb, sinb = np.cos(ang), np.sin(ang)
    cos64 = np.concatenate([cosb[0:16], cosb[0:16], cosb[16:32], cosb[16:32]], 0)
    sin64 = np.concatenate([-sinb[0:16], sinb[0:16], -sinb[16:32], sinb[16:32]], 0)
    cosA = np.concatenate([cos64, cos64], 0).astype(np.float16)   # [128, S]
    sinS = np.concatenate([sin64, sin64], 0).astype(np.float16)
    return cosA, sinS


def _host_prep(x, w_qkv, w_out):
    cosA, sinS = _host_tables()
    xT = np.ascontiguousarray(x.reshape(BS, D_MODEL).T).astype(np.float16)
    in_maps = []
    for c in range(N_CORES):
        h0, h1 = 2 * c, 2 * c + 1
        wq = np.concatenate([w_qkv[64 * h0:64 * h0 + 64][_PERM],
                             w_qkv[64 * h1:64 * h1 + 64][_PERM]], 0)
        wk = np.concatenate([w_qkv[1024 + 64 * h0:1024 + 64 * h0 + 64][_PERM],
                             w_qkv[1024 + 64 * h1:1024 + 64 * h1 + 64][_PERM]], 0)
        wv = np.concatenate([w_qkv[2048 + 64 * h0:2048 + 64 * h0 + 64],
                             w_qkv[2048 + 64 * h1:2048 + 64 * h1 + 64]], 0)
        w_all = np.concatenate([wq, wk, wv], 0)          # [384, 1024]
        in_maps.append({
            "xT": xT,
            "wqkvT": np.ascontiguousarray(w_all.T).astype(np.float16),
            "woutT": np.ascontiguousarray(
                w_out[:, 128 * c:128 * c + 128].T).astype(np.float16),
            "cosA": cosA,
            "sinS": sinS,
        })
    return in_maps


def _build_nc():
    import concourse.bacc as bacc
    import concourse.mybir as mybir
    from concourse import tile
    from concourse.masks import make_identity

    F16, F32 = mybir.dt.float16, mybir.dt.float32
    AF = mybir.ActivationFunctionType
    ALU = mybir.AluOpType

    nc = bacc.Bacc("TRN2", target_bir_lowering=False, debug=False,
                   num_devices=N_CORES)
    xT_d = nc.dram_tensor("xT", [D_MODEL, BS], F16, kind="ExternalInput")
    w_d = nc.dram_tensor("wqkvT", [D_MODEL, 384], F16, kind="ExternalInput")
    wo_d = nc.dram_tensor("woutT", [128, D_MODEL], F16, kind="ExternalInput")
    cos_d = nc.dram_tensor("cosA", [128, S], F16, kind="ExternalInput")
    sin_d = nc.dram_tensor("sinS", [128, S], F16, kind="ExternalInput")
    y_d = nc.dram_tensor("y", [D_MODEL, BS], F16, kind="ExternalOutput")

    with tile.TileContext(nc) as tc:
        with tc.tile_pool(name="const", bufs=1) as const, \
             tc.tile_pool(name="xr", bufs=3) as xrp, \
             tc.tile_pool(name="ur", bufs=2) as urp, \
             tc.tile_pool(name="ropet", bufs=2) as ropet, \
             tc.tile_pool(name="rot", bufs=2) as rotp, \
             tc.tile_pool(name="vr", bufs=2) as vrp, \
             tc.tile_pool(name="vone", bufs=2) as vonep, \
             tc.tile_pool(name="ep", bufs=5) as ep, \
             tc.tile_pool(name="outp", bufs=2) as outp, \
             tc.tile_pool(name="nrm", bufs=2) as nrm, \
             tc.tile_pool(name="ysb", bufs=3) as ysb, \
             tc.tile_pool(name="ps", bufs=1, space="PSUM") as ps:

            # ---- constants -------------------------------------------------
            w_r = const.tile([128, KC, 384], F16, tag="w")
            nc.sync.dma_start(
                w_r[:], w_d.ap().rearrange("(kc p) m -> p kc m", p=128))
            wo_r = const.tile([128, D_MODEL], F16, tag="wo")
            nc.sync.dma_start(wo_r[:], wo_d.ap())
            cosA = const.tile([128, S], F16, tag="cos")
            sinS = const.tile([128, S], F16, tag="sin")
            nc.sync.dma_start(cosA[:], cos_d.ap())
            nc.sync.dma_start(sinS[:], sin_d.ap())
            idf = const.tile([128, 128], F32, tag="idf")
            make_identity(nc, idf[:])
            id128 = const.tile([128, 128], F16, tag="idr")
            nc.vector.tensor_copy(id128[:], idf[:])
            orow_f = const.tile([1, 64], F32, tag="orowf")
            nc.vector.memset(orow_f[:], 1.0)
            orow_r = const.tile([1, 64], F16, tag="orow")
            nc.vector.tensor_copy(orow_r[:], orow_f[:])

            rot_tiles = {}
            vone_tiles = {}
            outT_tiles = {}

            def gen_qkv(b):
                """QKV projection + RoPE + V transpose for batch b."""
                c0 = b * S
                rot_q = rotp.tile([128, S], F16, tag="rq")
                rot_k = rotp.tile([128, S], F16, tag="rk")
                u_q = urp.tile([128, S], F16, tag="uq")
                u_k = urp.tile([128, S], F16, tag="uk")
                v_r = vrp.tile([128, S], F16, tag="vr")
                vone = vonep.tile([128, H_PER, NK, 65], F16, tag="vone")
                rot_tiles[b] = (rot_q, rot_k)
                vone_tiles[b] = vone
                nc.gpsimd.memset(vone[:, :, :, 64:65], 1.0)

                for half in range(2):           # n-chunk pairs (0,1), (2,3)
                    x_rs = []
                    for j in range(2):
                        nn = 2 * half + j
                        x_r = xrp.tile([128, KC, 512], F16, tag="xr",
                                       name=f"xr{j}")
                        for hf in range(2):
                            nc.sync.dma_start(
                                x_r[:, 4 * hf:4 * hf + 4, :],
                                xT_d.ap()[512 * hf:512 * hf + 512,
                                          c0 + nn * 512:c0 + (nn + 1) * 512]
                                .rearrange("(kc p) n -> p kc n", p=128))
                        x_rs.append(x_r)
                    yield

                    for qk in range(2):        # 0: Q, 1: K
                        pus = [ps.tile([128, 512], F32, tag="qkv", bufs=2,
                                       name=f"pu{j}")
                               for j in range(2)]
                        for kc in range(KC):
                            for j in range(2):
                                nc.tensor.matmul(
                                    pus[j][:],
                                    w_r[:, kc, 128 * qk:128 * qk + 128],
                                    x_rs[j][:, kc, :], start=(kc == 0),
                                    stop=(kc == KC - 1))
                            if kc == 3:
                                yield
                        yield
                        u_full = u_q if qk == 0 else u_k
                        for j in range(2):
                            nn = 2 * half + j
                            nc.vector.tensor_copy(
                                u_full[:, nn * 512:(nn + 1) * 512],
                                pus[j][:])
                        yield

                    # V chain
                    pvs = [ps.tile([128, 512], F32, tag="qkv", bufs=2,
                                   name=f"pvv{j}")
                           for j in range(2)]
                    for kc in range(KC):
                        for j in range(2):
                            nc.tensor.matmul(
                                pvs[j][:], w_r[:, kc, 256:384],
                                x_rs[j][:, kc, :], start=(kc == 0),
                                stop=(kc == KC - 1))
                        if kc == 3:
                            yield
                    yield
                    for j in range(2):
                        nn = 2 * half + j
                        nc.vector.tensor_copy(v_r[:, nn * 512:(nn + 1) * 512],
                                              pvs[j][:])
                    yield
                    # V transposes + merged vone copies
                    for kt in range(8 * half, 8 * half + 8):
                        pt = ps.tile([128, 2, 512], F16, tag="st", bufs=2,
                                     name="pt")
                        nc.tensor.transpose(
                            pt[:, 0, :128], v_r[:, kt * 128:(kt + 1) * 128],
                            id128[:])
                        nc.vector.tensor_copy(
                            vone[:, :, kt, :64],
                            pt[:, 0, :128].rearrange("p (h d) -> p h d", h=2))
                        if kt % 4 == 3:
                            yield

                # full-width RoPE, q/k chains interleaved
                sh_q = ropet.tile([128, S], F16, tag="swq")
                sh_k = ropet.tile([128, S], F16, tag="swk")
                nc.vector.stream_shuffle(sh_q[:], u_q[:], _SHUF_MASK)
                nc.vector.stream_shuffle(sh_k[:], u_k[:], _SHUF_MASK)
                yield
                m_q = ropet.tile([128, S], F16, tag="mq")
                m_k = ropet.tile([128, S], F16, tag="mk")
                nc.vector.tensor_mul(m_q[:], u_q[:], cosA[:])
                nc.vector.tensor_mul(m_k[:], u_k[:], cosA[:])
                yield
                t_q = ropet.tile([128, S], F16, tag="tq")
                t_k = ropet.tile([128, S], F16, tag="tk")
                nc.vector.tensor_mul(t_q[:], sh_q[:], sinS[:])
                nc.vector.tensor_mul(t_k[:], sh_k[:], sinS[:])
                yield
                nc.vector.tensor_add(rot_q[:], m_q[:], t_q[:])
                nc.vector.tensor_add(rot_k[:], m_k[:], t_k[:])
                yield

            def gen_attn(b):
                """Attention for batch b. Yields per ki step."""
                rot_q, rot_k = rot_tiles[b]
                vone = vone_tiles[b]
                outT = outp.tile([128, S], F16, tag="outT")
                outT_tiles[b] = outT

                for qc in range(NQ):
                    last_ki = 4 * qc + 3
                    pvt = [ps.tile([128, 512], F32, tag="pv", bufs=2,
                                   name=f"pv{hh}")
                           for hh in range(H_PER)]
                    pending = []   # (ki, e_p, nc_cols)

                    def emit_pv(ki, e_p, ncc):
                        for hh in range(H_PER):
                            nc.tensor.matmul(
                                pvt[hh][0:65, 512 - ncc:],
                                vone[:, hh, ki, :],
                                e_p[:, hh, :ncc], start=(ki == 0),
                                stop=(ki == last_ki))

                    for ki in range(last_ki + 1):
                        nc_cols = 512 - max(0, ki - 4 * qc) * 128
                        coff = max(qc * 512, ki * 128)
                        stp = ps.tile([128, 2, 512], F32, tag="st", bufs=2,
                                      name="stp")
                        for hh in range(H_PER):
                            nc.tensor.matmul(
                                stp[:, hh, :nc_cols],
                                rot_k[64 * hh:64 * hh + 64,
                                      ki * 128:(ki + 1) * 128],
                                rot_q[64 * hh:64 * hh + 64,
                                      coff:coff + nc_cols],
                                start=True, stop=True,
                                tile_position=(64 * hh, 0))
                        e_p = ep.tile([128, 2, 512], F16, tag="e")
                        nc.scalar.activation(e_p[:, :, :nc_cols],
                                             stp[:, :, :nc_cols],
                                             AF.Exp, scale=0.125)
                        if ki >= 4 * qc:   # diagonal block: zero k > q
                            nc.gpsimd.affine_select(
                                out=e_p[:, :, :128], in_=e_p[:, :, :128],
                                compare_op=ALU.is_ge, fill=0.0,
                                base=0, pattern=[[0, 2], [1, 128]],
                                channel_multiplier=-1)
                        pending.append((ki, e_p, nc_cols))
                        while len(pending) > 2:
                            emit_pv(*pending.pop(0))
                        yield
                    for args in pending:
                        emit_pv(*args)
                    pending = []

                    # normalize, hh chains stage-interleaved
                    dens = []
                    for hh in range(H_PER):
                        den1 = nrm.tile([1, 512], F16, tag="den",
                                        name=f"den{hh}")
                        nc.vector.tensor_copy(den1[:], pvt[hh][64:65, :])
                        dens.append(den1)
                    for hh in range(H_PER):
                        nc.tensor.matmul(pvt[hh][64:128, :], orow_r[:],
                                         dens[hh][:],
                                         start=True, stop=True,
                                         tile_position=(0, 64),
                                         skip_group_check=True)
                    rbs = []
                    for hh in range(H_PER):
                        rb = nrm.tile([64, 512], F16, tag="rb",
                                      name=f"rb{hh}")
                        with nc.allow_low_precision(
                                reason="fp16 softmax denominators"):
                            nc.vector.reciprocal(rb[:], pvt[hh][64:128, :])
                        rbs.append(rb)
                    for hh in range(H_PER):
                        nc.vector.tensor_mul(
                            outT[64 * hh:64 * hh + 64,
                                 qc * 512:(qc + 1) * 512],
                            pvt[hh][0:64, :], rbs[hh][:])
                    yield

            def gen_out(b):
                """Output projection y^T = wo^T @ out^T for batch b."""
                c0 = b * S
                outT = outT_tiles.pop(b)
                for ec in range(8):
                    for g in range(2):          # t pairs (0,1), (2,3)
                        pyp = ps.tile([128, 2, 512], F32, tag="st", bufs=2,
                                      name="pyp")
                        for t2 in range(2):
                            t = 2 * g + t2
                            nc.tensor.matmul(
                                pyp[:, t2, :],
                                wo_r[:, ec * 128:(ec + 1) * 128],
                                outT[:, t * 512:(t + 1) * 512],
                                start=True, stop=True)
                        y_sb = ysb.tile([128, 1024], F16, tag="ysb")
                        if (ec + g) % 2 == 0:
                            nc.vector.tensor_copy(
                                y_sb[:], pyp[:].rearrange("p a b -> p (a b)"))
                        else:
                            nc.scalar.copy(
                                y_sb[:], pyp[:].rearrange("p a b -> p (a b)"))
                        nc.sync.dma_start(
                            y_d.ap()[ec * 128:(ec + 1) * 128,
                                     c0 + 1024 * g:c0 + 1024 * (g + 1)],
                            y_sb[:])
                    yield
                rot_tiles.pop(b)
                vone_tiles.pop(b)

            def drain(g):
                for _ in g:
                    pass

            def interleave(main, filler):
                main, filler = iter(main), iter(filler)
                sentinel = object()
                while True:
                    m = next(main, sentinel)
                    f = next(filler, sentinel)
                    if m is sentinel and f is sentinel:
                        break

            import itertools
            drain(gen_qkv(0))
            for b in range(B):
                main = itertools.chain(gen_attn(b), gen_out(b))
                filler = gen_qkv(b + 1) if b + 1 < B else iter(())
                interleave(main, filler)
    nc.compile()
    return nc


def _get_nc():
    if "nc" not in _cached:
        _cached["nc"] = _build_nc()
    return _cached["nc"]


def kernel(x, w_qkv, w_out):
    from concourse.bass_utils import run_bass_kernel_spmd

    x = np.asarray(x, np.float32)
    w_qkv = np.asarray(w_qkv, np.float32)
    w_out = np.asarray(w_out, np.float32)
    in_maps = _host_prep(x, w_qkv, w_out)
    nc = _get_nc()
    res = run_bass_kernel_spmd(nc, in_maps, core_ids=list(range(N_CORES)),
                               trace=TRACE)
    global LAST_EXEC_NS, LAST_TRACE_PATH
    if res.exec_time_ns is not None:
        LAST_EXEC_NS = res.exec_time_ns
        if res.instructions_and_trace:
            LAST_TRACE_PATH = res.instructions_and_trace[1]
    yT = np.sum(np.stack([res.results[c]["y"].astype(np.float32)
                          for c in range(N_CORES)]), axis=0)
    return yT.T.reshape(B, S, D_MODEL).astype(np.float32)
